# revision 31
# baseline (speedup 1.0000x reference)
"""LocalAttention Bass kernel for Trainium2 (8 NeuronCores).

Problem: B=4 H=8 T=8192 D=64, window=128, look_backward=1, causal.
Sharding: pure (B*H) data parallelism — 32 heads / 8 cores = 4 heads each,
processed as 2 head-pairs so q/k DMAs use all 128 SBUF partitions.

Device algorithm (per head, per 128-token window w):
  S^T[k, q] = K_w' @ Q_w^T      (keys on partitions, so the softmax
                                 reduction over keys can ride the PV matmul)
  P = exp(S^T * D^-0.5) * causal01
  [O^T; r] = [V | 1]^T @ P      (ones column baked into V gives row-sums)
Host divides O^T by r and transposes back.

Performance history: fp32 baseline 305us -> v2 bf16 144us -> v3 114us.
v3.2 notes:
  - all matmul operands bf16 (fp32 matmul = 4 cyc/col vs 1 for bf16)
  - PE warm-up burst + per-group full-array (K=128,M=128) keep-warm
    matmuls: the HAM clock gate treats half-array activity (K=64 S
    matmuls / M=65 PV matmuls) as idle-ish and drops the PE to 1.2 GHz
    mid-run without them
  - PV matmuls merged via PSUM has_written semantics (start=True clears
    the whole bank; later start=False matmuls overwrite untouched
    elements, accumulate touched ones): 5 matmuls per group-head
  - software pipelining: group g+1's S matmuls are emitted BEFORE group
    g's PV so the in-order tensor queue never stalls the scalar engine;
    h0's sp is double-buffered (h1 hides behind h0's activation)
  - ONE packed DMA per (pair, chunk) loads q/k/v together (host packs
    them chunk-contiguously); stores are per-2-group and deferred one
    group so a store waiting on copies never blocks a load behind it
    in the sync queue FIFO
  - exp writes P directly as bf16; causal mask 1/3 vector + 2/3 gpsimd;
    output copy downcasts to bf16 (halves HBM writes)

PSUM budget (8 banks): sp0 x2 bufs (4) + sp1 x1 (2) + op x2 (2).

Packed input layout per (pair, chunk) [cols, all bf16]:
  [ Q^T (nw+1 windows) | K^T (nw windows) | Vh0 (nw+1 slots) | Vh1 ]
  Q^T has head A on partitions 0-63, head B on 64-127; one zero window
  appended globally (lookahead pad). V slot = [V(64) | ones] x 128 keys,
  slot 0 zeroed (look-back pad).
Output:
  outT [4, 65, 8192] bf16 — rows 0..63 unnormalized O^T, row 64 row-sums
"""

import numpy as np

B, H, T, D = 4, 8, 8192, 64
W = 128                     # window size
WIN = T // W                # 64 windows per head
NCORES = 8
BH = B * H                  # 32
BH_PER_CORE = BH // NCORES  # 4
NPAIR = BH_PER_CORE // 2    # 2 head pairs per core
G = 4                       # windows per softmax group (PSUM tile = [128, 1024])
# chunk sizes in windows: a tiny first chunk gets the pipeline started early
CHUNK_SIZES = [4, 12, 16, 16, 16]
assert sum(CHUNK_SIZES) == WIN
SCALE = float(D) ** -0.5
VS = D + 1                  # V slot width (64 values + ones column)

N_WARM = 32                 # PE warm-up matmuls (K=128, N=256 each)
KEEPWARM = 1                # full-array dummies per group to hold K=8/8

# packed column layout per chunk: (qoff, koff, voff0, voff1, total)
_CHCOL = []
_coff = 0
for _nw in CHUNK_SIZES:
    _q = (_nw + 1) * W
    _k = _nw * W
    _v = (_nw + 1) * VS
    _CHCOL.append((_coff, _coff + _q, _coff + _q + _k,
                   _coff + _q + _k + _v, _q + _k + 2 * _v))
    _coff += _q + _k + 2 * _v
TOTCOL = _coff

_nc_cache = {}
last_perf = None


def _build_nc(skip=()):
    import concourse.tile as tile
    from concourse import bacc
    from concourse import mybir
    from contextlib import ExitStack

    f32 = mybir.dt.float32
    bf16 = mybir.dt.bfloat16
    Exp = mybir.ActivationFunctionType.Exp
    mult = mybir.AluOpType.mult

    nc = bacc.Bacc()
    qkv = nc.dram_tensor("qkv", [NPAIR, W, TOTCOL], bf16,
                         kind="ExternalInput")
    mask = nc.dram_tensor("mask01", [W, W], bf16, kind="ExternalInput")
    outT = nc.dram_tensor("outT", [BH_PER_CORE, D + 1, T], bf16,
                          kind="ExternalOutput")

    with tile.TileContext(nc) as tc, ExitStack() as ctx:
        cpool = ctx.enter_context(tc.tile_pool(name="cpool", bufs=1))
        iopool = ctx.enter_context(tc.tile_pool(name="iopool", bufs=1))
        opool = ctx.enter_context(tc.tile_pool(name="opool", bufs=3))
        ppool = ctx.enter_context(tc.tile_pool(name="ppool", bufs=4))
        spsum = ctx.enter_context(tc.tile_pool(name="spsum", bufs=3,
                                               space="PSUM"))
        opsum = ctx.enter_context(tc.tile_pool(name="opsum", bufs=2,
                                               space="PSUM"))

        mtile = cpool.tile([W, W], bf16)
        nc.sync.dma_start(mtile[:], mask[:])
        z128 = cpool.tile([W, W], bf16)      # P for the all-masked pad window
        nc.vector.memset(z128[:], 0.0)

        mm = nc.tensor.matmul

        # --- PE warm-up: flip the HAM clock gate to 2.4 GHz while the ---
        # --- first input DMA is in flight. K=128/M=128 so the FULL    ---
        # --- array lights up (half-array activity doesn't count).     ---
        if "warm" not in skip and N_WARM:
            wz = cpool.tile([W, 2 * W], bf16)
            nc.vector.memset(wz[:], 0.0)
            wps = spsum.tile([W, G * 2 * W], f32, tag="sp")
            for _ in range(N_WARM):
                mm(wps[:, 0:2 * W], z128[:, :], wz[:],
                   start=True, stop=True)

        def s_phase(ck, g, sps):
            """S^T pairblock matmuls for one group: h0 block then h1.

            Two leading full-array dummies keep the HAM busy-detector fed
            (overwritten by the real i=0 matmul)."""
            qc, kc = ck["qc"], ck["kc"]
            w0 = g * G
            if "warm" not in skip:
                for _ in range(KEEPWARM):
                    mm(sps[0][:, 0:W], z128[:, :], z128[:, :],
                       start=True, stop=True)
            for h in range(2):
                hb = h * 64
                for i in range(G):
                    wl = w0 + i
                    mm(sps[h][:, i * 256:(i + 1) * 256],
                       kc[hb:hb + 64, wl * W:(wl + 1) * W],
                       qc[hb:hb + 64, wl * W:(wl + 2) * W],
                       start=True, stop=True)

        gidx = 0

        def consume_phase(p, ck, g, gp, sps, pt_prev, ocs):
            """exp + mask + merged PV + output copy for one group."""
            nonlocal gidx
            w0 = g * G
            for h in range(2):
                vc = ck["vcs"][h]
                pt = ppool.tile([W, G * 2 * W], bf16, tag=f"pt{h}",
                                name=f"pt{h}")
                if "exp" not in skip:
                    nc.scalar.activation(pt[:], sps[h][:], Exp, scale=SCALE)

                # causal mask on T1 blocks (cols 0,256,512,768)
                pt3 = pt[:].rearrange("p (g x) -> p g x", x=2 * W)
                t1 = pt3[:, :, 0:W]
                mb = mtile[:, None, :].to_broadcast([W, G, W])
                if "mask" not in skip:
                    if gidx % 3 == 0:
                        nc.vector.tensor_tensor(t1, t1, mb, mult)
                    else:
                        nc.gpsimd.tensor_tensor(t1, t1, mb, mult)

                # merged PV + row-sums: 5 matmuls instead of 8
                op = opsum.tile([D + 1, G * W], f32, tag="op", name="op")
                if "pv" not in skip:
                    if pt_prev[h] is not None:
                        t0src = pt_prev[h][:, G * 256 - W:G * 256]
                    else:
                        t0src = z128[:]
                    mm(op[:, 0:W], vc[:, w0 * VS:(w0 + 1) * VS],
                       t0src, start=True, stop=False)
                    for j in range(1, G):
                        mm(op[:, (j - 1) * W:(j + 1) * W],
                           vc[:, (w0 + j) * VS:(w0 + j + 1) * VS],
                           pt[:, (j - 1) * 256:j * 256],
                           start=False, stop=False)
                    mm(op[:, (G - 1) * W:G * W],
                       vc[:, (w0 + G) * VS:(w0 + G + 1) * VS],
                       pt[:, (G - 1) * 256:(G - 1) * 256 + W],
                       start=False, stop=True)

                # per-2-group output staging tiles
                half = (gp % 2) * G * W
                if gp % 2 == 0:
                    ocs[h] = opool.tile([D + 1, 2 * G * W], bf16,
                                        tag="oc", name="oc")
                if "ocopy" not in skip:
                    nc.vector.tensor_copy(
                        ocs[h][:, half:half + G * W], op[:])
                pt_prev[h] = pt
                gidx += 1

        def load_chunk(p, c):
            ws = sum(CHUNK_SIZES[:c])
            nw = CHUNK_SIZES[c]
            qoff, koff, voff0, voff1, ncol = _CHCOL[c]
            qk = iopool.tile([W, ncol], bf16, tag=f"qk{c}", name="qk")
            if "loads" not in skip:
                nc.sync.dma_start(qk[:], qkv[p, :, qoff:qoff + ncol])
            qc = qk[:, 0:(nw + 1) * W]
            kc = qk[:, koff - qoff:koff - qoff + nw * W]
            vcs = [qk[:, voff0 - qoff:voff0 - qoff + (nw + 1) * VS],
                   qk[:, voff1 - qoff:voff1 - qoff + (nw + 1) * VS]]
            return dict(qc=qc, kc=kc, vcs=vcs, c=c, p=p)

        # global schedule
        sched = [(p, c, g) for p in range(NPAIR)
                 for c in range(len(CHUNK_SIZES))
                 for g in range(CHUNK_SIZES[c] // G)]
        n = len(sched)
        gpp = n // NPAIR     # groups per pair
        chunks = {}
        pt_prev_by_p = {p: [None, None] for p in range(NPAIR)}
        ocs_by_p = {p: [None, None] for p in range(NPAIR)}
        sp_of = {}
        pending_store = []

        def ensure_chunk(i):
            if i >= n:
                return
            p, c, g = sched[i]
            if (p, c) not in chunks:
                chunks[(p, c)] = load_chunk(p, c)

        def emit_s(i):
            p, c, g = sched[i]
            ensure_chunk(i)
            ck = chunks[(p, c)]
            sp0 = spsum.tile([W, G * 2 * W], f32, tag="sp", name="sp0")
            sp1 = spsum.tile([W, G * 2 * W], f32, tag="sp", name="sp1")
            sps = [sp0, sp1]
            sp_of[i] = sps
            if "smm" not in skip:
                s_phase(ck, g, sps)

        def flush_stores():
            while pending_store:
                pp, gp, tiles = pending_store.pop(0)
                c0 = (gp - 1) * G * W
                for h in range(2):
                    nc.sync.dma_start(
                        outT[2 * pp + h, :, c0:c0 + 2 * G * W], tiles[h])

        ensure_chunk(0)
        emit_s(0)
        for i in range(n):
            p, c, g = sched[i]
            # prefetch the chunk needed 3 groups ahead; loads go to the
            # sync queue BEFORE any pending store can block it
            ensure_chunk(i + 3)
            flush_stores()
            if i + 1 < n:
                emit_s(i + 1)
            ck = chunks[(p, c)]
            gp = i - p * gpp
            consume_phase(p, ck, g, gp, sp_of.pop(i), pt_prev_by_p[p],
                          ocs_by_p[p])
            if gp % 2 == 1 and "store" not in skip:
                pending_store.append((p, gp, list(ocs_by_p[p])))
        flush_stores()
    nc.finalize()
    return nc


def _prep_core_inputs(q2, k2, v2, core):
    import ml_dtypes
    bf16 = ml_dtypes.bfloat16
    s0 = core * BH_PER_CORE
    qkv = np.zeros((NPAIR, W, TOTCOL), bf16)
    mask01 = (np.arange(W)[:, None] <= np.arange(W)[None, :]).astype(bf16)
    for p in range(NPAIR):
        # full per-pair Q^T (padded with one zero window) / K^T / V slots
        qT = np.zeros((W, (WIN + 1) * W), bf16)
        kT = np.zeros((W, T), bf16)
        for h in range(2):
            bh = s0 + 2 * p + h
            qT[h * 64:(h + 1) * 64, :T] = q2[bh].T.astype(bf16)
            kT[h * 64:(h + 1) * 64, :] = k2[bh].T.astype(bf16)
        vps = []
        for h in range(2):
            bh = s0 + 2 * p + h
            vr = v2[bh].reshape(WIN, W, D).transpose(1, 0, 2)   # [W,WIN,D]
            vph = np.zeros((W, WIN + 1, VS), bf16)
            vph[:, 1:, :D] = vr.astype(bf16)
            vph[:, :, D] = 1.0
            vps.append(vph.reshape(W, (WIN + 1) * VS))
        ws = 0
        for c, nw in enumerate(CHUNK_SIZES):
            qoff, koff, voff0, voff1, ncol = _CHCOL[c]
            qkv[p, :, qoff:qoff + (nw + 1) * W] = \
                qT[:, ws * W:(ws + nw + 1) * W]
            qkv[p, :, koff:koff + nw * W] = kT[:, ws * W:(ws + nw) * W]
            for h, voff in ((0, voff0), (1, voff1)):
                qkv[p, :, voff:voff + (nw + 1) * VS] = \
                    vps[h][:, ws * VS:(ws + nw + 1) * VS]
            ws += nw
    return {"qkv": qkv, "mask01": mask01}


def kernel(q, k, v, _trace=False):
    global last_perf
    from concourse.bass_utils import run_bass_kernel_spmd

    q = np.ascontiguousarray(np.asarray(q), dtype=np.float32)
    k = np.ascontiguousarray(np.asarray(k), dtype=np.float32)
    v = np.ascontiguousarray(np.asarray(v), dtype=np.float32)
    q2 = q.reshape(BH, T, D)
    k2 = k.reshape(BH, T, D)
    v2 = v.reshape(BH, T, D)

    if "nc" not in _nc_cache:
        _nc_cache["nc"] = _build_nc()
    nc = _nc_cache["nc"]

    in_maps = [_prep_core_inputs(q2, k2, v2, core) for core in range(NCORES)]
    res = run_bass_kernel_spmd(
        nc, in_maps, core_ids=list(range(NCORES)), trace=_trace)
    last_perf = res

    outs = []
    for core in range(NCORES):
        ot = np.asarray(res.results[core]["outT"], dtype=np.float32)
        o = ot[:, :D, :] / ot[:, D:D + 1, :]           # normalize
        outs.append(o.transpose(0, 2, 1))              # [4, T, 64]
    full = np.concatenate(outs, axis=0)                # [32, T, 64]
    return full.reshape(B, H, T, D)


# revision 36
# speedup vs baseline: 1.0130x; 1.0130x over previous
"""LocalAttention Bass kernel for Trainium2 (8 NeuronCores).

Problem: B=4 H=8 T=8192 D=64, window=128, look_backward=1, causal.
Sharding: pure (B*H) data parallelism — 32 heads / 8 cores = 4 heads each,
processed as 2 head-pairs so q/k DMAs use all 128 SBUF partitions.

Device algorithm (per head, per 128-token window w):
  S^T[k, q] = K_w' @ Q_w^T      (keys on partitions, so the softmax
                                 reduction over keys can ride the PV matmul)
  P = exp(S^T * D^-0.5) * causal01
  [O^T; r] = [V | 1]^T @ P      (ones column baked into V gives row-sums)
Host divides O^T by r and transposes back.

Performance history: fp32 baseline 305us -> v2 bf16 144us -> v3 114us.
v3.2 notes:
  - all matmul operands bf16 (fp32 matmul = 4 cyc/col vs 1 for bf16)
  - PE warm-up burst + per-group full-array (K=128,M=128) keep-warm
    matmuls: the HAM clock gate treats half-array activity (K=64 S
    matmuls / M=65 PV matmuls) as idle-ish and drops the PE to 1.2 GHz
    mid-run without them
  - PV matmuls merged via PSUM has_written semantics (start=True clears
    the whole bank; later start=False matmuls overwrite untouched
    elements, accumulate touched ones): 5 matmuls per group-head
  - software pipelining: group g+1's S matmuls are emitted BEFORE group
    g's PV so the in-order tensor queue never stalls the scalar engine;
    h0's sp is double-buffered (h1 hides behind h0's activation)
  - ONE packed DMA per (pair, chunk) loads q/k/v together (host packs
    them chunk-contiguously); stores are per-4-group and deferred one
    group so a store waiting on copies never blocks a load behind it
    in the sync queue FIFO
  - exp writes P directly as bf16; causal mask 1/3 vector + 2/3 gpsimd;
    output copy downcasts to bf16 (halves HBM writes)

PSUM budget (8 banks): sp0 x2 bufs (4) + sp1 x1 (2) + op x2 (2).

Packed input layout per (pair, chunk) [cols, all bf16]:
  [ Q^T (nw+1 windows) | K^T (nw windows) | Vh0 (nw+1 slots) | Vh1 ]
  Q^T has head A on partitions 0-63, head B on 64-127; one zero window
  appended globally (lookahead pad). V slot = [V(64) | ones] x 128 keys,
  slot 0 zeroed (look-back pad).
Output:
  outT [4, 65, 8192] bf16 — rows 0..63 unnormalized O^T, row 64 row-sums
"""

import numpy as np

B, H, T, D = 4, 8, 8192, 64
W = 128                     # window size
WIN = T // W                # 64 windows per head
NCORES = 8
BH = B * H                  # 32
BH_PER_CORE = BH // NCORES  # 4
NPAIR = BH_PER_CORE // 2    # 2 head pairs per core
G = 4                       # windows per softmax group (PSUM tile = [128, 1024])
# chunk sizes in windows: a tiny first chunk gets the pipeline started early
CHUNK_SIZES = [4, 12, 16, 16, 16]
assert sum(CHUNK_SIZES) == WIN
SCALE = float(D) ** -0.5
VS = D + 1                  # V slot width (64 values + ones column)

N_WARM = 40                 # PE warm-up matmuls (K=128, N=256 each)
KEEPWARM = 1                # full-array dummies per group to hold K=8/8

# packed column layout per chunk: (qoff, koff, voff0, voff1, total)
_CHCOL = []
_coff = 0
for _nw in CHUNK_SIZES:
    _q = (_nw + 1) * W
    _k = _nw * W
    _v = (_nw + 1) * VS
    _CHCOL.append((_coff, _coff + _q, _coff + _q + _k,
                   _coff + _q + _k + _v, _q + _k + 2 * _v))
    _coff += _q + _k + 2 * _v
TOTCOL = _coff

_nc_cache = {}
last_perf = None


def _build_nc(skip=()):
    import concourse.tile as tile
    from concourse import bacc
    from concourse import mybir
    from contextlib import ExitStack

    f32 = mybir.dt.float32
    bf16 = mybir.dt.bfloat16
    Exp = mybir.ActivationFunctionType.Exp
    mult = mybir.AluOpType.mult

    nc = bacc.Bacc()
    qkv = nc.dram_tensor("qkv", [NPAIR, W, TOTCOL], bf16,
                         kind="ExternalInput")
    mask = nc.dram_tensor("mask01", [W, W], bf16, kind="ExternalInput")
    outT = nc.dram_tensor("outT", [BH_PER_CORE, D + 1, T], bf16,
                          kind="ExternalOutput")

    with tile.TileContext(nc) as tc, ExitStack() as ctx:
        cpool = ctx.enter_context(tc.tile_pool(name="cpool", bufs=1))
        iopool = ctx.enter_context(tc.tile_pool(name="iopool", bufs=1))
        opool = ctx.enter_context(tc.tile_pool(name="opool", bufs=3))
        ppool = ctx.enter_context(tc.tile_pool(name="ppool", bufs=4))
        spsum = ctx.enter_context(tc.tile_pool(name="spsum", bufs=3,
                                               space="PSUM"))
        opsum = ctx.enter_context(tc.tile_pool(name="opsum", bufs=2,
                                               space="PSUM"))

        mtile = cpool.tile([W, W], bf16)
        nc.sync.dma_start(mtile[:], mask[:])
        z128 = cpool.tile([W, W], bf16)      # P for the all-masked pad window
        nc.vector.memset(z128[:], 0.0)

        mm = nc.tensor.matmul

        # --- PE warm-up: flip the HAM clock gate to 2.4 GHz while the ---
        # --- first input DMA is in flight. K=128/M=128 so the FULL    ---
        # --- array lights up (half-array activity doesn't count).     ---
        if "warm" not in skip and N_WARM:
            wz = cpool.tile([W, 2 * W], bf16)
            nc.vector.memset(wz[:], 0.0)
            wps = spsum.tile([W, G * 2 * W], f32, tag="sp")
            for _ in range(N_WARM):
                mm(wps[:, 0:2 * W], z128[:, :], wz[:],
                   start=True, stop=True)

        def s_phase(ck, g, sps):
            """S^T pairblock matmuls for one group: h0 block then h1.

            Two leading full-array dummies keep the HAM busy-detector fed
            (overwritten by the real i=0 matmul)."""
            qc, kc = ck["qc"], ck["kc"]
            w0 = g * G
            if "warm" not in skip:
                for _ in range(KEEPWARM):
                    mm(sps[0][:, 0:W], z128[:, :], z128[:, :],
                       start=True, stop=True)
            for h in range(2):
                hb = h * 64
                for i in range(G):
                    wl = w0 + i
                    mm(sps[h][:, i * 256:(i + 1) * 256],
                       kc[hb:hb + 64, wl * W:(wl + 1) * W],
                       qc[hb:hb + 64, wl * W:(wl + 2) * W],
                       start=True, stop=True)

        gidx = 0

        def consume_phase(p, ck, g, gp, sps, pt_prev, ocs):
            """exp + mask + merged PV + output copy for one group."""
            nonlocal gidx
            w0 = g * G
            for h in range(2):
                vc = ck["vcs"][h]
                pt = ppool.tile([W, G * 2 * W], bf16, tag=f"pt{h}",
                                name=f"pt{h}")
                if "exp" not in skip:
                    nc.scalar.activation(pt[:], sps[h][:], Exp, scale=SCALE)

                # causal mask on T1 blocks (cols 0,256,512,768)
                pt3 = pt[:].rearrange("p (g x) -> p g x", x=2 * W)
                t1 = pt3[:, :, 0:W]
                mb = mtile[:, None, :].to_broadcast([W, G, W])
                if "mask" not in skip:
                    if gidx % 3 == 0:
                        nc.vector.tensor_tensor(t1, t1, mb, mult)
                    else:
                        nc.gpsimd.tensor_tensor(t1, t1, mb, mult)

                # merged PV + row-sums: 5 matmuls instead of 8
                op = opsum.tile([D + 1, G * W], f32, tag="op", name="op")
                if "pv" not in skip:
                    if pt_prev[h] is not None:
                        t0src = pt_prev[h][:, G * 256 - W:G * 256]
                    else:
                        t0src = z128[:]
                    mm(op[:, 0:W], vc[:, w0 * VS:(w0 + 1) * VS],
                       t0src, start=True, stop=False)
                    for j in range(1, G):
                        mm(op[:, (j - 1) * W:(j + 1) * W],
                           vc[:, (w0 + j) * VS:(w0 + j + 1) * VS],
                           pt[:, (j - 1) * 256:j * 256],
                           start=False, stop=False)
                    mm(op[:, (G - 1) * W:G * W],
                       vc[:, (w0 + G) * VS:(w0 + G + 1) * VS],
                       pt[:, (G - 1) * 256:(G - 1) * 256 + W],
                       start=False, stop=True)

                # per-4-group output staging -> fewer sync-queue stores
                quarter = (gp % 4) * G * W
                if gp % 4 == 0:
                    ocs[h] = opool.tile([D + 1, 4 * G * W], bf16,
                                        tag="oc", name="oc")
                if "ocopy" not in skip:
                    nc.vector.tensor_copy(
                        ocs[h][:, quarter:quarter + G * W], op[:])
                pt_prev[h] = pt
                gidx += 1

        def load_chunk(p, c):
            ws = sum(CHUNK_SIZES[:c])
            nw = CHUNK_SIZES[c]
            qoff, koff, voff0, voff1, ncol = _CHCOL[c]
            qk = iopool.tile([W, ncol], bf16, tag=f"qk{c}", name="qk")
            if "loads" not in skip:
                nc.sync.dma_start(qk[:], qkv[p, :, qoff:qoff + ncol])
            qc = qk[:, 0:(nw + 1) * W]
            kc = qk[:, koff - qoff:koff - qoff + nw * W]
            vcs = [qk[:, voff0 - qoff:voff0 - qoff + (nw + 1) * VS],
                   qk[:, voff1 - qoff:voff1 - qoff + (nw + 1) * VS]]
            return dict(qc=qc, kc=kc, vcs=vcs, c=c, p=p)

        # global schedule
        sched = [(p, c, g) for p in range(NPAIR)
                 for c in range(len(CHUNK_SIZES))
                 for g in range(CHUNK_SIZES[c] // G)]
        n = len(sched)
        gpp = n // NPAIR     # groups per pair
        chunks = {}
        pt_prev_by_p = {p: [None, None] for p in range(NPAIR)}
        ocs_by_p = {p: [None, None] for p in range(NPAIR)}
        sp_of = {}
        pending_store = []

        def ensure_chunk(i):
            if i >= n:
                return
            p, c, g = sched[i]
            if (p, c) not in chunks:
                chunks[(p, c)] = load_chunk(p, c)

        def emit_s(i):
            p, c, g = sched[i]
            ensure_chunk(i)
            ck = chunks[(p, c)]
            sp0 = spsum.tile([W, G * 2 * W], f32, tag="sp", name="sp0")
            sp1 = spsum.tile([W, G * 2 * W], f32, tag="sp", name="sp1")
            sps = [sp0, sp1]
            sp_of[i] = sps
            if "smm" not in skip:
                s_phase(ck, g, sps)

        def flush_stores():
            while pending_store:
                pp, gp, tiles = pending_store.pop(0)
                c0 = (gp - 3) * G * W
                for h in range(2):
                    nc.sync.dma_start(
                        outT[2 * pp + h, :, c0:c0 + 4 * G * W], tiles[h])

        ensure_chunk(0)
        emit_s(0)
        for i in range(n):
            p, c, g = sched[i]
            # prefetch the chunk needed 3 groups ahead; loads go to the
            # sync queue BEFORE any pending store can block it
            ensure_chunk(i + 3)
            flush_stores()
            if i + 1 < n:
                emit_s(i + 1)
            ck = chunks[(p, c)]
            gp = i - p * gpp
            consume_phase(p, ck, g, gp, sp_of.pop(i), pt_prev_by_p[p],
                          ocs_by_p[p])
            if gp % 4 == 3 and "store" not in skip:
                pending_store.append((p, gp, list(ocs_by_p[p])))
        flush_stores()
    nc.finalize()
    return nc


def _prep_core_inputs(q2, k2, v2, core):
    import ml_dtypes
    bf16 = ml_dtypes.bfloat16
    s0 = core * BH_PER_CORE
    qkv = np.zeros((NPAIR, W, TOTCOL), bf16)
    mask01 = (np.arange(W)[:, None] <= np.arange(W)[None, :]).astype(bf16)
    for p in range(NPAIR):
        # full per-pair Q^T (padded with one zero window) / K^T / V slots
        qT = np.zeros((W, (WIN + 1) * W), bf16)
        kT = np.zeros((W, T), bf16)
        for h in range(2):
            bh = s0 + 2 * p + h
            qT[h * 64:(h + 1) * 64, :T] = q2[bh].T.astype(bf16)
            kT[h * 64:(h + 1) * 64, :] = k2[bh].T.astype(bf16)
        vps = []
        for h in range(2):
            bh = s0 + 2 * p + h
            vr = v2[bh].reshape(WIN, W, D).transpose(1, 0, 2)   # [W,WIN,D]
            vph = np.zeros((W, WIN + 1, VS), bf16)
            vph[:, 1:, :D] = vr.astype(bf16)
            vph[:, :, D] = 1.0
            vps.append(vph.reshape(W, (WIN + 1) * VS))
        ws = 0
        for c, nw in enumerate(CHUNK_SIZES):
            qoff, koff, voff0, voff1, ncol = _CHCOL[c]
            qkv[p, :, qoff:qoff + (nw + 1) * W] = \
                qT[:, ws * W:(ws + nw + 1) * W]
            qkv[p, :, koff:koff + nw * W] = kT[:, ws * W:(ws + nw) * W]
            for h, voff in ((0, voff0), (1, voff1)):
                qkv[p, :, voff:voff + (nw + 1) * VS] = \
                    vps[h][:, ws * VS:(ws + nw + 1) * VS]
            ws += nw
    return {"qkv": qkv, "mask01": mask01}


def kernel(q, k, v, _trace=False):
    global last_perf
    from concourse.bass_utils import run_bass_kernel_spmd

    q = np.ascontiguousarray(np.asarray(q), dtype=np.float32)
    k = np.ascontiguousarray(np.asarray(k), dtype=np.float32)
    v = np.ascontiguousarray(np.asarray(v), dtype=np.float32)
    q2 = q.reshape(BH, T, D)
    k2 = k.reshape(BH, T, D)
    v2 = v.reshape(BH, T, D)

    if "nc" not in _nc_cache:
        _nc_cache["nc"] = _build_nc()
    nc = _nc_cache["nc"]

    in_maps = [_prep_core_inputs(q2, k2, v2, core) for core in range(NCORES)]
    res = run_bass_kernel_spmd(
        nc, in_maps, core_ids=list(range(NCORES)), trace=_trace)
    last_perf = res

    outs = []
    for core in range(NCORES):
        ot = np.asarray(res.results[core]["outT"], dtype=np.float32)
        o = ot[:, :D, :] / ot[:, D:D + 1, :]           # normalize
        outs.append(o.transpose(0, 2, 1))              # [4, T, 64]
    full = np.concatenate(outs, axis=0)                # [32, T, 64]
    return full.reshape(B, H, T, D)
